# revision 10
# baseline (speedup 1.0000x reference)
"""AttentionBlockWithSkipConnection Trainium2 kernel.

Full inputs -> full output. Data-parallel over batch B=8 across 8 cores.
Each core computes one batch: GroupNorm -> qkv 1x1conv -> full 4096x4096
attention -> proj 1x1conv -> skip add.

Layout strategy: channel-major ("transposed") throughout the middle of the
pipeline so every matmul contracts over the partition dim with no transposes
of the big attention matrix:
  h^T [C, N]          (C=256 as 2 partition-chunks of 128)
  qkv^T = w_qkv.T @ h^T               (w_qkv already stored [C, 3C] = lhsT)
  logits^T[k,q] = (K^T).T @ Q^T       (both operands channel-major)
  expT = exp(logits^T / 16)           (softmax denominator = partition sums,
                                       accumulated on DVE + ones-matmul)
  o_un^T = V.T @ expT                  (V token-major via 64 PE transposes)
  proj_un^T = w_proj.T @ o_un^T
  out = transpose(proj_un^T) * (1/colsum) + b_proj + x   (scale folded into
                                       the per-partition ACT copy after the
                                       PE transpose back to token-major)
"""

import numpy as np

import concourse.bacc as bacc
import concourse.bass as bass
import concourse.mybir as mybir
import concourse.tile as tile
from concourse.bass_utils import run_bass_kernel_spmd
from concourse.masks import make_identity

N_CORES = 8
B, H, W, C = 8, 64, 64, 256
N = H * W  # 4096 tokens
G = 32  # groups
GS = C // G  # 8 channels per group
EPS = 1e-5
CC = C // 128  # 2 channel chunks
QT = 512  # q tile (free dim of logits/attnv matmuls)
NQ = N // QT  # 8
NK = N // 128  # 32 k tiles
F32 = mybir.dt.float32

# Matmul input dtype for the heavy matmuls. float32r streams at ~4x the rate
# of float32 on the PE for free dims >= 256 (reduced internal precision).
USE_F32R = True


def _mm(ap):
    if USE_F32R:
        return ap.bitcast(mybir.dt.float32r)
    return ap


def _rw(ap):
    """Round-on-write: engine writes through this AP round to fp32r, which
    the walrus verifier requires for anything consumed by an fp32r matmul."""
    if USE_F32R:
        return ap.bitcast(mybir.dt.float32r)
    return ap


def _build():
    nc = bacc.Bacc(
        "TRN2",
        target_bir_lowering=False,
        debug=False,
        enable_asserts=True,
        num_devices=N_CORES,
    )
    x_d = nc.dram_tensor("x", [N, C], F32, kind="ExternalInput")
    gns_d = nc.dram_tensor("gn_scale", [C], F32, kind="ExternalInput")
    gnb_d = nc.dram_tensor("gn_bias", [C], F32, kind="ExternalInput")
    wq_d = nc.dram_tensor("w_qkv", [C, 3 * C], F32, kind="ExternalInput")
    bq_d = nc.dram_tensor("b_qkv", [3 * C], F32, kind="ExternalInput")
    wp_d = nc.dram_tensor("w_proj", [C, C], F32, kind="ExternalInput")
    bp_d = nc.dram_tensor("b_proj", [C], F32, kind="ExternalInput")
    out_d = nc.dram_tensor("out", [N, C], F32, kind="ExternalOutput")

    # group-aggregation masks: gA averages 8 consecutive partitions into one
    # group row; gB broadcasts group rows back to their 128 channels.
    gA_np = np.zeros((128, 16), np.float32)
    gB_np = np.zeros((16, 128), np.float32)
    for p in range(128):
        gA_np[p, p // GS] = 1.0 / GS
        gB_np[p // GS, p] = 1.0
    gA_d = nc.inline_tensor(gA_np, "gA")
    gB_d = nc.inline_tensor(gB_np, "gB")

    with tile.TileContext(nc) as tc:
        _body(tc, x_d, gns_d, gnb_d, wq_d, bq_d, wp_d, bp_d, out_d, gA_d, gB_d)
    nc.compile()
    return nc


def _body(tc, x_d, gns_d, gnb_d, wq_d, bq_d, wp_d, bp_d, out_d, gA_d, gB_d):
    nc = tc.nc
    x_tok = x_d.ap().rearrange("(nt p) c -> p nt c", p=128)  # [128, 32, 256]
    out_tok = out_d.ap().rearrange("(nt p) c -> p nt c", p=128)

    with (
        tc.tile_pool(name="consts", bufs=1) as consts,
        tc.tile_pool(name="psum_tr", bufs=2, space="PSUM") as psum_tr,
        tc.tile_pool(name="psum_mm", bufs=2, space="PSUM") as psum_mm,
        tc.tile_pool(name="psum_acc", bufs=1, space="PSUM") as psum_acc,
        tc.tile_pool(name="psum_small", bufs=2, space="PSUM") as psum_small,
        tc.tile_pool(name="dram_scratch", bufs=2, space="DRAM") as dram_scratch,
        tc.tile_pool(name="qkvT", bufs=1) as qkvT_pool,
    ):
        # ---- constants ----
        ident = consts.tile([128, 128], F32)
        make_identity(nc, ident)
        ones_col = consts.tile([128, 1], F32)
        nc.vector.memset(ones_col, 1.0)
        eps_col = consts.tile([128, 1], F32)
        nc.vector.memset(eps_col, EPS)
        gA = consts.tile([128, 16], F32)
        nc.sync.dma_start(out=gA, in_=gA_d.ap())
        gB = consts.tile([16, 128], F32)
        nc.sync.dma_start(out=gB, in_=gB_d.ap())
        wq_stage = consts.tile([128, CC, 3 * C], F32)
        nc.sync.dma_start(
            out=wq_stage, in_=wq_d.ap().rearrange("(cc p) d -> p cc d", p=128)
        )
        wq = consts.tile([128, CC, 3 * C], F32)
        nc.vector.tensor_copy(out=_rw(wq), in_=wq_stage)
        wp_stage = consts.tile([128, CC, C], F32)
        nc.sync.dma_start(
            out=wp_stage, in_=wp_d.ap().rearrange("(cc p) d -> p cc d", p=128)
        )
        wp = consts.tile([128, CC, C], F32)
        nc.vector.tensor_copy(out=_rw(wp), in_=wp_stage)
        bq = consts.tile([128, 6], F32)
        nc.sync.dma_start(out=bq, in_=bq_d.ap().rearrange("(m p) -> p m", p=128))
        bp_rep = consts.tile([128, C], F32)
        nc.sync.dma_start(
            out=bp_rep,
            in_=bass.AP(tensor=bp_d, offset=0, ap=[[0, 128], [1, C]]),
        )
        gns = consts.tile([128, CC], F32)
        nc.sync.dma_start(out=gns, in_=gns_d.ap().rearrange("(cc p) -> p cc", p=128))
        gnb = consts.tile([128, CC], F32)
        nc.sync.dma_start(out=gnb, in_=gnb_d.ap().rearrange("(cc p) -> p cc", p=128))

        qkvT = qkvT_pool.tile([128, 6, N], F32)  # 96KB/partition

        # ---- phase A: load x, transpose to channel-major, groupnorm ----
        with (
            tc.tile_pool(name="xcm", bufs=1) as xcm_pool,
            tc.tile_pool(name="xtm", bufs=1) as xtm_pool,
            tc.tile_pool(name="gn_stats", bufs=2) as gn_stats,
        ):
            x_cm = xcm_pool.tile([128, CC, N], F32)  # 32KB/partition
            x_tm = xtm_pool.tile([128, 32, C], F32)  # 32KB/partition
            nc.sync.dma_start(out=x_tm, in_=x_tok)
            for nt in range(32):
                for cc in range(CC):
                    ps = psum_tr.tile([128, 128], F32, tag="tr")
                    nc.tensor.transpose(
                        ps, x_tm[:, nt, cc * 128 : (cc + 1) * 128], ident
                    )
                    nc.vector.tensor_copy(
                        out=_rw(x_cm[:, cc, nt * 128 : (nt + 1) * 128]), in_=ps
                    )

            # per-channel stats over the 4096 tokens
            ab = gn_stats.tile([128, CC, 2], F32)  # (a, b) per channel
            for cc in range(CC):
                stats = gn_stats.tile([128, 8, 6], F32, tag="stats")
                xg = x_cm[:, cc, :].rearrange("p (s f) -> p s f", f=512)
                for s in range(8):
                    nc.vector.bn_stats(out=stats[:, s, :], in_=xg[:, s, :])
                mv = gn_stats.tile([128, 2], F32, tag="mv")
                nc.vector.bn_aggr(out=mv, in_=stats)
                # mv2 = (mean, E[x^2])
                mv2 = gn_stats.tile([128, 2], F32, tag="mv2")
                nc.vector.tensor_copy(out=mv2[:, 0:1], in_=mv[:, 0:1])
                nc.vector.tensor_mul(out=mv2[:, 1:2], in0=mv[:, 0:1], in1=mv[:, 0:1])
                nc.vector.tensor_add(out=mv2[:, 1:2], in0=mv2[:, 1:2], in1=mv[:, 1:2])
                # aggregate to 16 group rows, then broadcast back to channels
                gp = psum_small.tile([16, 2], F32, tag="small")
                nc.tensor.matmul(gp, lhsT=gA, rhs=mv2, start=True, stop=True)
                gp_sb = gn_stats.tile([16, 2], F32, tag="gp_sb")
                nc.vector.tensor_copy(out=gp_sb, in_=gp)
                chs = psum_small.tile([128, 2], F32, tag="small")
                nc.tensor.matmul(chs, lhsT=gB, rhs=gp_sb, start=True, stop=True)
                chs_sb = gn_stats.tile([128, 2], F32, tag="chs_sb")
                nc.vector.tensor_copy(out=chs_sb, in_=chs)
                # var = E[x^2] - mean^2 ; rstd = 1/sqrt(var+eps)
                var = gn_stats.tile([128, 1], F32, tag="var")
                msq = gn_stats.tile([128, 1], F32, tag="msq")
                nc.vector.tensor_mul(out=msq, in0=chs_sb[:, 0:1], in1=chs_sb[:, 0:1])
                nc.vector.tensor_sub(out=var, in0=chs_sb[:, 1:2], in1=msq)
                nc.scalar.activation(
                    out=var,
                    in_=var,
                    func=mybir.ActivationFunctionType.Sqrt,
                    bias=eps_col,
                )
                rstd = gn_stats.tile([128, 1], F32, tag="rstd")
                nc.vector.reciprocal(out=rstd, in_=var)
                # a = rstd*gn_scale ; b = gn_bias - mean*a
                nc.vector.tensor_mul(
                    out=ab[:, cc, 0:1], in0=rstd, in1=gns[:, cc : cc + 1]
                )
                nc.vector.tensor_mul(out=msq, in0=chs_sb[:, 0:1], in1=ab[:, cc, 0:1])
                nc.vector.tensor_sub(
                    out=ab[:, cc, 1:2], in0=gnb[:, cc : cc + 1], in1=msq
                )
            # normalize in place: h = x*a + b
            for cc in range(CC):
                nc.vector.tensor_scalar(
                    out=_rw(x_cm[:, cc, :]),
                    in0=x_cm[:, cc, :],
                    scalar1=ab[:, cc, 0:1],
                    scalar2=ab[:, cc, 1:2],
                    op0=mybir.AluOpType.mult,
                    op1=mybir.AluOpType.add,
                )

            # ---- phase B: qkv^T = w_qkv.T @ h^T  (+ b_qkv) ----
            for m in range(6):
                for qt in range(NQ):
                    ps = psum_mm.tile([128, QT], F32, tag="mm")
                    for cc in range(CC):
                        nc.tensor.matmul(
                            ps,
                            lhsT=_mm(wq[:, cc, m * 128 : (m + 1) * 128]),
                            rhs=_mm(x_cm[:, cc, qt * QT : (qt + 1) * QT]),
                            start=(cc == 0),
                            stop=(cc == CC - 1),
                        )
                    nc.scalar.activation(
                        out=_rw(qkvT[:, m, qt * QT : (qt + 1) * QT]),
                        in_=ps,
                        func=mybir.ActivationFunctionType.Identity,
                        bias=bq[:, m : m + 1],
                    )

        # ---- phase C: V token-major via PE transposes ----
        with tc.tile_pool(name="vtm", bufs=1) as vtm_pool:
            v_tm = vtm_pool.tile([128, 32, C], F32)
            for nt in range(32):
                for cc in range(CC):
                    ps = psum_tr.tile([128, 128], F32, tag="tr")
                    nc.tensor.transpose(
                        ps, qkvT[:, 4 + cc, nt * 128 : (nt + 1) * 128], ident
                    )
                    nc.vector.tensor_copy(
                        out=_rw(v_tm[:, nt, cc * 128 : (cc + 1) * 128]), in_=ps
                    )

            # ---- phase D: attention + proj + skip, per q tile ----
            with (
                tc.tile_pool(name="expp", bufs=3) as expp,
                tc.tile_pool(name="accp", bufs=2) as accp,
                tc.tile_pool(name="owork", bufs=2) as owork,
            ):
                for qt in range(NQ):
                    av_ps = [
                        psum_acc.tile(
                            [128, QT], F32, tag=f"av_ps{cc}", name=f"av_ps{cc}"
                        )
                        for cc in range(CC)
                    ]
                    expacc = accp.tile([128, QT], F32, tag="expacc")
                    for kt in range(NK):
                        lg = psum_mm.tile([128, QT], F32, tag="mm", name="lg")
                        for cc in range(CC):
                            nc.tensor.matmul(
                                lg,
                                lhsT=_mm(qkvT[:, 2 + cc, kt * 128 : (kt + 1) * 128]),
                                rhs=_mm(qkvT[:, cc, qt * QT : (qt + 1) * QT]),
                                start=(cc == 0),
                                stop=(cc == CC - 1),
                            )
                        expT = expp.tile([128, QT], F32, tag="expT")
                        nc.scalar.activation(
                            out=_rw(expT),
                            in_=lg,
                            func=mybir.ActivationFunctionType.Exp,
                            scale=1.0 / 16.0,
                        )
                        if kt == 0:
                            nc.vector.tensor_copy(out=expacc, in_=expT)
                        else:
                            nc.vector.tensor_add(out=expacc, in0=expacc, in1=expT)
                        for cc in range(CC):
                            nc.tensor.matmul(
                                av_ps[cc],
                                lhsT=_mm(v_tm[:, kt, cc * 128 : (cc + 1) * 128]),
                                rhs=_mm(expT),
                                start=(kt == 0),
                                stop=(kt == NK - 1),
                            )
                    # softmax denominator: column sums of expT = ones @ expacc
                    cs = psum_small.tile([1, QT], F32, tag="small")
                    nc.tensor.matmul(cs, lhsT=ones_col, rhs=expacc, start=True, stop=True)
                    cs_sb = owork.tile([1, QT], F32, tag="cs_sb")
                    nc.vector.tensor_copy(out=cs_sb, in_=cs)
                    cs_dram = dram_scratch.tile([QT], F32)
                    nc.sync.dma_start(out=cs_dram, in_=cs_sb)
                    recip = owork.tile([128, 4], F32, tag="recip")
                    nc.sync.dma_start(
                        out=recip, in_=cs_dram.rearrange("(qq p) -> p qq", p=128)
                    )
                    nc.vector.reciprocal(out=recip, in_=recip)

                    # proj_un^T = w_proj.T @ o_un^T
                    av_sb = owork.tile([128, CC, QT], F32, tag="av_sb")
                    for cc in range(CC):
                        nc.scalar.copy(out=_rw(av_sb[:, cc, :]), in_=av_ps[cc])
                    pj_sb = owork.tile([128, CC, QT], F32, tag="pj_sb")
                    for dc in range(CC):
                        ps = psum_mm.tile([128, QT], F32, tag="mm", name="pj_ps")
                        for cc in range(CC):
                            nc.tensor.matmul(
                                ps,
                                lhsT=_mm(wp[:, cc, dc * 128 : (dc + 1) * 128]),
                                rhs=_mm(av_sb[:, cc, :]),
                                start=(cc == 0),
                                stop=(cc == CC - 1),
                            )
                        nc.scalar.copy(out=pj_sb[:, dc, :], in_=ps)

                    # back to token-major; fold 1/colsum into the copy scale
                    out_sb = owork.tile([128, 4, C], F32, tag="out_sb")
                    x_re = owork.tile([128, 4, C], F32, tag="x_re")
                    nc.sync.dma_start(
                        out=x_re, in_=x_tok[:, qt * 4 : (qt + 1) * 4, :]
                    )
                    for qq in range(4):
                        for dc in range(CC):
                            ps = psum_tr.tile([128, 128], F32, tag="tr", name="ps_out")
                            nc.tensor.transpose(
                                ps, pj_sb[:, dc, qq * 128 : (qq + 1) * 128], ident
                            )
                            nc.scalar.activation(
                                out=out_sb[:, qq, dc * 128 : (dc + 1) * 128],
                                in_=ps,
                                func=mybir.ActivationFunctionType.Copy,
                                scale=recip[:, qq : qq + 1],
                            )
                        nc.vector.tensor_add(
                            out=out_sb[:, qq, :], in0=out_sb[:, qq, :], in1=bp_rep
                        )
                        nc.vector.tensor_add(
                            out=out_sb[:, qq, :], in0=out_sb[:, qq, :], in1=x_re[:, qq, :]
                        )
                    nc.sync.dma_start(
                        out=out_tok[:, qt * 4 : (qt + 1) * 4, :], in_=out_sb
                    )


_NC = None


def _get_nc():
    global _NC
    if _NC is None:
        _NC = _build()
    return _NC


def kernel(x, gn_scale, gn_bias, w_qkv, b_qkv, w_proj, b_proj):
    nc = _get_nc()
    x = np.asarray(x, dtype=np.float32).reshape(B, N, C)
    shared = {
        "gn_scale": np.asarray(gn_scale, np.float32),
        "gn_bias": np.asarray(gn_bias, np.float32),
        "w_qkv": np.ascontiguousarray(w_qkv, np.float32),
        "b_qkv": np.asarray(b_qkv, np.float32),
        "w_proj": np.ascontiguousarray(w_proj, np.float32),
        "b_proj": np.asarray(b_proj, np.float32),
    }
    in_maps = [{"x": np.ascontiguousarray(x[i]), **shared} for i in range(N_CORES)]
    res = run_bass_kernel_spmd(nc, in_maps, list(range(N_CORES)))
    out = np.stack([res.results[i]["out"] for i in range(N_CORES)])
    return out.reshape(B, H, W, C)
